# revision 2
# baseline (speedup 1.0000x reference)
"""EMD loss kernel for Trainium2 (8 NeuronCores, pure data parallel).

Computes out[b] = sum_t (cumsum(x-y, axis=1)[b, t])^2 for x, y [131072, 256] f32.

Per-core plan (16384 rows each, no cross-core communication):
  - x and y are packed host-side into one [2, 16384, 256] DRAM parameter so
    each streaming chunk is ONE DMA instruction, and x/y arrive together.
  - Chunks are loaded via SWDGE (gpsimd) DMA with an f32 -> fp16 cast in the
    SDMA datapath: HBM read traffic is unchanged (the roofline), but all
    on-chip operands become 16-bit, which makes the DVE 2x packing modes
    available and halves SBUF write traffic.
  - View the shard as [128 partitions, 128 row-blocks, 256 bins]; per
    row-block a VectorE tensor_tensor_scan computes the running CDF
    difference state = (x_t + state) - y_t in ONE instruction (fp32 state
    internally, fp16 out).
  - The per-row sum of squares is split across engines to balance load:
    most blocks use ScalarE activation(Square, accum_out=...) (one ACTIVATE
    + one ACTIVATION_READ_ACCUMULATOR), and an interleaved subset uses
    VectorE (tensor_tensor mult + tensor_reduce) so neither engine falls
    behind the DMA stream rate.
  - The output store is split in two so most of it overlaps the last blocks'
    compute; only a thin final slice is serialized at the end.
  - Tail chunks taper (8,4,2,1,1 blocks) in dedicated pool slots so trailing
    compute after the last DMA is ~1 row-block.
"""

import numpy as np

from concourse import bacc, bass, mybir
from concourse.bass_utils import run_bass_kernel_spmd
from concourse.tile import TileContext

N_CORES = 8
B = 131072
BINS = 256
ROWS = B // N_CORES  # 16384 rows per core
P = 128  # SBUF partitions
N_BLK = ROWS // P  # 128 row-blocks per core (one row per partition each)
HEAD = [8] * 14  # main-pool streaming chunks
CHUNK_SLOT = 8  # main io pool slot size in row-blocks
IO_BUFS = 8
TAIL = [8, 4, 2, 1, 1]  # dedicated slots each
CHUNKS = HEAD + TAIL
assert sum(CHUNKS) == N_BLK
C_BUFS = 16
SQ_BUFS = 8

# Row-sum-of-squares engine split: ~1/6 of blocks go to DVE (mult+reduce),
# the rest to ACT (Square+accum). Interleaved so the load stays balanced.
DVE_SUM_EVERY = 6  # block i uses DVE when i % DVE_SUM_EVERY == 2

# Split output store: first slice issued as soon as its columns are final.
OUT_SPLIT = 120

F32 = mybir.dt.float32
F16 = mybir.dt.float16


def build_nc() -> bass.Bass:
    nc = bacc.Bacc()

    xy = nc.declare_dram_parameter("xy", [2, ROWS, BINS], F32, isOutput=False)
    out = nc.declare_dram_parameter("out", [ROWS], F32, isOutput=True)

    # [128, 2, N_BLK * BINS]; partition p holds rows p*N_BLK .. p*N_BLK+N_BLK-1
    xyv = xy[:].rearrange("z (p n) d -> p z (n d)", p=P)
    ov = out[:].rearrange("(p n) -> p n", p=P)  # [128, N_BLK]

    with (
        TileContext(nc) as tc,
        tc.tile_pool(name="io", bufs=IO_BUFS) as io_pool,
        tc.tile_pool(name="iotail", bufs=1) as tail_pool,
        tc.tile_pool(name="cdf", bufs=C_BUFS) as c_pool,
        tc.tile_pool(name="res", bufs=1) as res_pool,
        tc.tile_pool(name="sq", bufs=SQ_BUFS, space="PSUM") as sq_pool,
        tc.tile_pool(name="sq16", bufs=SQ_BUFS) as sq16_pool,
    ):
        out_sb = res_pool.tile([P, N_BLK], F32)

        # Warm the ACT Square table at t=0 so the ~2.7us table load overlaps
        # the first input DMAs instead of stalling the first real activation.
        warm = res_pool.tile([P, 1], F32, tag="warm")
        warm2 = res_pool.tile([P, 1], F32, tag="warm2")
        nc.vector.memset(warm[:], 0)
        nc.scalar.activation(
            out=warm2[:],
            in_=warm[:],
            func=mybir.ActivationFunctionType.Square,
        )

        blk0 = 0
        for ci, tsz in enumerate(CHUNKS):
            if ci < len(HEAD):
                slot = CHUNK_SLOT
                xyt = io_pool.tile(
                    [P, 2 * slot * BINS], F16, tag="xyt", name=f"xyt{ci}"
                )
            else:
                slot = tsz
                xyt = tail_pool.tile(
                    [P, 2 * slot * BINS], F16, tag=f"tail{ci}", name=f"xyt{ci}"
                )
            # [128, 2, tsz*256] view of the slot: x at free offset 0, y at
            # slot*BINS — matches the DRAM [p, z, f] chunk below. SWDGE casts
            # f32 -> fp16 inline in the SDMA datapath.
            xyt3 = xyt[:].rearrange("p (z f) -> p z f", z=2)[:, :, : tsz * BINS]
            lo, hi = blk0 * BINS, (blk0 + tsz) * BINS
            nc.gpsimd.dma_start(out=xyt3, in_=xyv[:, :, lo:hi])
            for t in range(tsz):
                col = blk0 + t
                xoff = t * BINS
                yoff = slot * BINS + t * BINS
                c = c_pool.tile([P, BINS], F16)
                nc.vector.tensor_tensor_scan(
                    out=c[:],
                    data0=xyt[:, xoff : xoff + BINS],
                    data1=xyt[:, yoff : yoff + BINS],
                    initial=0.0,
                    op0=mybir.AluOpType.add,
                    op1=mybir.AluOpType.subtract,
                )
                if col % DVE_SUM_EVERY == 2:
                    sq16 = sq16_pool.tile([P, BINS], F16)
                    nc.vector.tensor_tensor(
                        out=sq16[:],
                        in0=c[:],
                        in1=c[:],
                        op=mybir.AluOpType.mult,
                    )
                    nc.vector.tensor_reduce(
                        out=out_sb[:, col : col + 1],
                        in_=sq16[:],
                        axis=mybir.AxisListType.X,
                        op=mybir.AluOpType.add,
                    )
                else:
                    sq = sq_pool.tile([P, BINS], F32)
                    nc.scalar.activation(
                        out=sq[:],
                        in_=c[:],
                        func=mybir.ActivationFunctionType.Square,
                        accum_out=out_sb[:, col : col + 1],
                    )
            blk0 += tsz
        nc.sync.dma_start(out=ov[:, :OUT_SPLIT], in_=out_sb[:, :OUT_SPLIT])
        nc.sync.dma_start(out=ov[:, OUT_SPLIT:], in_=out_sb[:, OUT_SPLIT:])
    nc.finalize()
    return nc


_NC = None


def _get_nc() -> bass.Bass:
    global _NC
    if _NC is None:
        _NC = build_nc()
    return _NC


def kernel(x: np.ndarray, y: np.ndarray) -> np.ndarray:
    assert x.shape == (B, BINS) and y.shape == (B, BINS), (x.shape, y.shape)
    x = np.ascontiguousarray(x, dtype=np.float32)
    y = np.ascontiguousarray(y, dtype=np.float32)
    in_maps = []
    for i in range(N_CORES):
        sl = slice(i * ROWS, (i + 1) * ROWS)
        in_maps.append({"xy": np.stack([x[sl], y[sl]])})
    res = run_bass_kernel_spmd(_get_nc(), in_maps, list(range(N_CORES)))
    return np.concatenate([m["out"] for m in res.results])


# revision 3
# speedup vs baseline: 1.1261x; 1.1261x over previous
"""EMD loss kernel for Trainium2 (8 NeuronCores, pure data parallel).

Computes out[b] = sum_t (cumsum(x-y, axis=1)[b, t])^2 for x, y [131072, 256] f32.

Per-core plan (16384 rows each, no cross-core communication):
  - x and y are packed host-side into one [2, 16384, 256] DRAM parameter so
    each streaming chunk is ONE DMA instruction, and x/y arrive together.
  - Chunks are loaded via SWDGE (gpsimd) DMA with an f32 -> fp16 cast in the
    SDMA datapath: HBM read traffic is unchanged (the roofline), SBUF write
    traffic halves.
  - View the shard as [128 partitions, 128 row-blocks, 256 bins]; per
    row-block a SINGLE custom DVE instruction (EMD_FUSED_ANT, registered
    below via the concourse custom-DVE Spec DSL) computes
        state = cumsum(x - y)   (zero-bubble inclusive scan, fp32 ALU)
        out   = state^2         (elementwise, discarded)
        accum_out = sum(state^2)  (the per-row EMD loss)
    so the whole kernel is one DVE op per block and is DMA-bound; the
    scalar/tensor/gpsimd engines do nothing but DMA dispatch.
  - Tail chunks taper (8,4,2,1,1 blocks) in dedicated pool slots so trailing
    compute after the last DMA is ~1 row-block; the output store is split so
    most of it overlaps the tail compute.
"""

import numpy as np

from concourse import bacc, bass, dve_ops, mybir
from concourse.bass_utils import run_bass_kernel_spmd
from concourse.dve_spec import AluOp, Spec, Src0, Src1, Zero, lower, scan, sq
from concourse.dve_spec import _has_src1 as has_src1
from concourse.dve_uop import DveOpSpec
from concourse.tile import TileContext


def _emd_ref(in0, in1, c0, c1, c2):
    cdf = np.cumsum(in0.astype(np.float32) - in1.astype(np.float32), axis=-1)
    b = (cdf * cdf).astype(np.float32)
    return b, b.reshape(b.shape[0], -1).sum(axis=-1, keepdims=True)


def _register_emd_op() -> "dve_ops.DveOp":
    """Define and register the fused scan+square+reduce custom-DVE op.

    accum_out = sum_k (cumsum_k(in0 - in1))^2, out = the squared running
    state (discarded by the caller). Registered exactly like the stock ops in
    dve_ops.OPS: next free opcode row, sha pinned from this build's lower().
    """
    name = "EMD_FUSED_ANT"
    for op in dve_ops.OPS:
        if op.name == name:
            return op
    spec = Spec(
        body=sq(scan(AluOp.ADD, Src0 - Src1)),
        accum=AluOp.ADD,
        accum_init=Zero,
        reference=_emd_ref,
    )
    row = dve_ops._CUSTOM_DVE_ROW_BASE + len(dve_ops.OPS)
    assert row < 0x20, "custom-DVE opcode rows exhausted"
    shas = {}
    for ver in ("v3", "v4"):
        s = DveOpSpec(
            name=name, opcode=row, uops=lower(spec, ver=ver), rd1_en=has_src1(spec)
        )
        shas[ver] = s.sha(ver)
    op = dve_ops.DveOp(name, spec, subdim=False, uops_sha=shas)
    dve_ops.OPS.append(op)
    dve_ops._SUB_OPCODE_FOR_NAME[name] = row
    dve_ops.CUSTOM_DVE_SPECS[name] = spec
    return op


EMD_OP = _register_emd_op()

N_CORES = 8
B = 131072
BINS = 256
ROWS = B // N_CORES  # 16384 rows per core
P = 128  # SBUF partitions
N_BLK = ROWS // P  # 128 row-blocks per core (one row per partition each)
HEAD = [8] * 14  # main-pool streaming chunks
CHUNK_SLOT = 8  # main io pool slot size in row-blocks
IO_BUFS = 8
TAIL = [8, 4, 2, 1, 1]  # dedicated slots each
CHUNKS = HEAD + TAIL
assert sum(CHUNKS) == N_BLK
SQ_BUFS = 8

# Split output store: first slice issued as soon as its columns are final.
OUT_SPLIT = 120

F32 = mybir.dt.float32
F16 = mybir.dt.float16


def build_nc() -> bass.Bass:
    nc = bacc.Bacc()

    xy = nc.declare_dram_parameter("xy", [2, ROWS, BINS], F32, isOutput=False)
    out = nc.declare_dram_parameter("out", [ROWS], F32, isOutput=True)

    # [128, 2, N_BLK * BINS]; partition p holds rows p*N_BLK .. p*N_BLK+N_BLK-1
    xyv = xy[:].rearrange("z (p n) d -> p z (n d)", p=P)
    ov = out[:].rearrange("(p n) -> p n", p=P)  # [128, N_BLK]

    with (
        TileContext(nc) as tc,
        tc.tile_pool(name="io", bufs=IO_BUFS) as io_pool,
        tc.tile_pool(name="iotail", bufs=1) as tail_pool,
        tc.tile_pool(name="res", bufs=1) as res_pool,
        tc.tile_pool(name="sq", bufs=SQ_BUFS) as sq_pool,
    ):
        out_sb = res_pool.tile([P, N_BLK], F32)

        blk0 = 0
        for ci, tsz in enumerate(CHUNKS):
            if ci < len(HEAD):
                slot = CHUNK_SLOT
                xyt = io_pool.tile(
                    [P, 2 * slot * BINS], F16, tag="xyt", name=f"xyt{ci}"
                )
            else:
                slot = tsz
                xyt = tail_pool.tile(
                    [P, 2 * slot * BINS], F16, tag=f"tail{ci}", name=f"xyt{ci}"
                )
            # [128, 2, tsz*256] view of the slot: x at free offset 0, y at
            # slot*BINS — matches the DRAM [p, z, f] chunk below. SWDGE casts
            # f32 -> fp16 inline in the SDMA datapath.
            xyt3 = xyt[:].rearrange("p (z f) -> p z f", z=2)[:, :, : tsz * BINS]
            lo, hi = blk0 * BINS, (blk0 + tsz) * BINS
            nc.gpsimd.dma_start(out=xyt3, in_=xyv[:, :, lo:hi])
            for t in range(tsz):
                col = blk0 + t
                xoff = t * BINS
                yoff = slot * BINS + t * BINS
                sqt = sq_pool.tile([P, BINS], F16)
                nc.vector._custom_dve(
                    EMD_OP,
                    out=sqt[:],
                    in0=xyt[:, xoff : xoff + BINS],
                    in1=xyt[:, yoff : yoff + BINS],
                    accum_out=out_sb[:, col : col + 1],
                )
            blk0 += tsz
        nc.sync.dma_start(out=ov[:, :OUT_SPLIT], in_=out_sb[:, :OUT_SPLIT])
        nc.sync.dma_start(out=ov[:, OUT_SPLIT:], in_=out_sb[:, OUT_SPLIT:])
    nc.finalize()
    return nc


_NC = None


def _get_nc() -> bass.Bass:
    global _NC
    if _NC is None:
        _NC = build_nc()
    return _NC


def kernel(x: np.ndarray, y: np.ndarray) -> np.ndarray:
    assert x.shape == (B, BINS) and y.shape == (B, BINS), (x.shape, y.shape)
    x = np.ascontiguousarray(x, dtype=np.float32)
    y = np.ascontiguousarray(y, dtype=np.float32)
    in_maps = []
    for i in range(N_CORES):
        sl = slice(i * ROWS, (i + 1) * ROWS)
        in_maps.append({"xy": np.stack([x[sl], y[sl]])})
    res = run_bass_kernel_spmd(_get_nc(), in_maps, list(range(N_CORES)))
    return np.concatenate([m["out"] for m in res.results])
